# revision 72
# baseline (speedup 1.0000x reference)
"""Trainium2 Bass kernel for nn_ClassifierGCN (GCN conv -> z@z^T -> MLP -> sigmoid).

Contract: kernel(**inputs) takes the FULL unsharded inputs (numpy), distributes
across 8 NeuronCores internally, and returns the FULL output (numpy, f32).

Strategy (8 cores), v2:
  - Host: dense edge-count matrix C[src, dst] in fp8 (exact small ints),
    column-sharded 640 dst nodes/core; D^-1/2 ships as two tiny f32 tensors.
  - Phase A (bf16, as v1): h' = dinv_src * (x @ Wg) for ALL nodes,
    aggT = h'.T @ C_slice, zT = relu(dinv_dst * aggT + bg)  [128 x 640].
  - Phase B: G = z z^T is SYMMETRIC -> compute only the upper-triangle 16x16
    blocks (15 of 25) straight from zT, pack to 3840 els/graph in fp8;
    host pre-folds W1 rows (W1[rc]+W1[cr]) so the math stays exact.
    One AllGather moves the packed fp8 G of all 64 graphs (246KB).
  - Phase C: fc1 in transposed orientation (packed-W1 stationary / fp8
    DoubleRow, G moving) -> y1T tiles land in exactly the layout fc2 needs;
    fc2 graph-major (y1T stationary fp8, W2 fp8 DoubleRow moving).
    Weights ship fp8 scaled x64; scales fold into relu (exact pow2) and the
    final sigmoid's activation scale (1/4096). One ReduceScatter(add, bf16)
    returns each graph's logits to its home core; + b2, sigmoid, store.
"""

import numpy as np
import ml_dtypes

import bass_rust
import concourse.bass as bass
import concourse.mybir as mybir
import concourse.tile as tile
from concourse.bass_utils import run_bass_kernel_spmd
from concourse.masks import make_identity
from concourse.tile_rust import add_dep_helper

# Problem shapes (hardcoded per contract).
N_NEURONS = 80
TBL = 256
LATENT = 128
N_GRAPHS = 64
N_NODES = 5120
N_CORES = 8
GPC = N_GRAPHS // N_CORES          # graphs per core = 8
DPC = N_NODES // N_CORES           # dst nodes per core = 640
N2 = N_NEURONS * N_NEURONS         # 6400
HID = 2 * N2                       # 12800
HS = HID // N_CORES                # hidden slice per core = 1600

DT = mybir.dt.bfloat16
F8 = mybir.dt.float8e4
F32 = mybir.dt.float32
NP_BF16 = ml_dtypes.bfloat16
NP_F8 = ml_dtypes.float8_e4m3

K_TILES_NODES = N_NODES // 128     # 40

# Symmetric-G block packing: 5x5 grid of 16x16 blocks, keep upper triangle.
NB = 5
BS = 16                            # N_NEURONS // NB
BLOCKS = [(i, j) for i in range(NB) for j in range(i, NB)]   # 15 blocks
NBLK = len(BLOCKS)                 # 15
BSQ = BS * BS                      # 256
GP = NBLK * BSQ                    # 3840 packed length per graph
GKT = GP // 128                    # 30 k-tiles
GKP = GKT // 2                     # 15 DoubleRow pairs
M_TILES_HS = [(t * 128, min(128, HS - t * 128)) for t in range((HS + 127) // 128)]  # 13
W2_PAIRS = 6                       # 1600 = 6*256 + 64
N_CHUNKS_1600 = [(0, 512), (512, 512), (1024, 512), (1536, 64)]
DR = mybir.MatmulPerfMode.DoubleRow
W1SCALE = 64.0
W2SCALE = 64.0
P2SCALE = W1SCALE * W2SCALE        # 4096
KP_NODES = K_TILES_NODES // 2      # 20 node-tile pairs for DoubleRow


def _fix_excess_waits(nc):
    """This container's walrus rejects >1 sem-wait on CTRL-class instructions.
    Tile's end-of-context Drain can carry several; move the excess onto NoOp
    carriers inserted just before, same engine, program order preserved."""
    n_fix = 0
    for f in nc.m.functions:
        for bb in f.blocks:
            out, changed = [], False
            for inst in bb.instructions:
                si = inst.sync_info
                waits = list(si.on_wait) if si is not None and si.on_wait else []
                if len(waits) > 1:
                    for w in waits[:-1]:
                        nop = mybir.InstNoOp(name=f"I-waitfix-{n_fix}", ins=[], outs=[])
                        n_fix += 1
                        nop.engine = inst.engine
                        nop.sync_info = bass_rust.SyncInfo(on_wait=[w], on_update=[])
                        out.append(nop)
                    si.on_wait = waits[-1:]
                    changed = True
                out.append(inst)
            if changed:
                bb.instructions = out
    return n_fix


def build_nc():
    nc = bass.Bass(num_devices=N_CORES)

    # xs: host-packed [(p), (pair, i, tbl)] fp8 of dinv_src-prescaled x, so
    # that xs[:, p, :, :] is directly a DoubleRow lhsT pair of node-tiles.
    xs = nc.dram_tensor("xs", [128, KP_NODES * 2 * TBL], F8,
                        kind="ExternalInput")
    wg = nc.dram_tensor("wg", [TBL, LATENT], DT, kind="ExternalInput")
    bg = nc.dram_tensor("bg", [LATENT, 1], F32, kind="ExternalInput")
    ats = nc.dram_tensor("ats", [N_NODES, DPC], F8, kind="ExternalInput")
    dinv_d = nc.dram_tensor("dinv_d", [128, DPC], F32, kind="ExternalInput")
    w1p = nc.dram_tensor("w1p", [GP, HS], F8, kind="ExternalInput")
    b1s = nc.dram_tensor("b1s", [128, len(M_TILES_HS)], F32, kind="ExternalInput")
    w2s = nc.dram_tensor("w2s", [HS, N2], F8, kind="ExternalInput")
    b2s = nc.dram_tensor("b2s", [128, N2 // 16], DT, kind="ExternalInput")
    y = nc.dram_tensor("y", [GPC, N2], F32, kind="ExternalOutput")

    RG = [list(range(N_CORES))]
    NMT = len(M_TILES_HS)          # 13

    with tile.TileContext(nc) as tc:
        with (
            # Weight pools first: their SBUF ranges never overlap phase pools,
            # so the prefetch streams from t=0.
            tc.tile_pool(name="w1pool", bufs=1) as w1pool,
            tc.tile_pool(name="w2pool", bufs=2) as w2pool,
            tc.tile_pool(name="w2lpool", bufs=2) as w2lpool,
            tc.tile_pool(name="const", bufs=1) as constp,
            tc.tile_pool(name="persist", bufs=1) as persist,
            tc.tile_pool(name="dram", bufs=1, space="DRAM") as dramp,
        ):
            # ---- DMA orchestration ----
            # The cost model's DMA engines behave as one FIFO: order of entry
            # is order of service. Early loads (x, ats, consts) go unchained
            # on the two HWDGE rings. The big weight stream is chained
            # (depth 2, completion deps) on the SP ring so the queue never
            # holds more than ~2 weight transfers — mid-stream DMAs (G
            # AllGather bounce, y partials), issued on the gpsimd SWDGE ring,
            # then slot in with bounded latency. The Activation ring carries
            # no chained DMAs so its SEQ stays free for activations/copies.
            with (
                tc.tile_pool(name="xa", bufs=1) as xap,
                tc.tile_pool(name="atp", bufs=6) as atp,
                tc.tile_pool(name="xtcs", bufs=1) as xtcp,
                tc.tile_pool(name="xtps", bufs=1, space="PSUM") as xtps,
                tc.tile_pool(name="aggps", bufs=1, space="PSUM") as aggps,
            ):
                at = []
                at0 = atp.tile([128, 4, DPC], F8, tag="at0", bufs=1)
                nc.scalar.dma_start(
                    at0[:], ats[0:512, :].rearrange("(a b) c -> b a c", a=4))
                at.append(at0)
                xsb = xap.tile([128, KP_NODES, 2, TBL], F8, tag="xs")
                nc.sync.dma_start(
                    xsb[:], xs[:, :].rearrange("p (k i t) -> p k i t",
                                               k=KP_NODES, i=2))
                at_dmas = []
                for i in range(1, 10):
                    t = atp.tile([128, 4, DPC], F8, tag=f"at{i % 6}", bufs=1)
                    eng = nc.sync if i % 2 == 0 else nc.scalar
                    h = eng.dma_start(
                        t[:], ats[i * 512:(i + 1) * 512, :].rearrange(
                            "(a b) c -> b a c", a=4))
                    at_dmas.append(h)
                    at.append(t)

                wg_sb = constp.tile([128, 2, LATENT], DT)
                nc.scalar.dma_start(
                    wg_sb[:], wg[:, :].rearrange("(a b) c -> b a c", a=2))
                dinv_d_sb = constp.tile([128, DPC], F32)
                nc.scalar.dma_start(dinv_d_sb[:], dinv_d[:, :])
                bg_sb = constp.tile([LATENT, 1], F32)
                nc.scalar.dma_start(bg_sb[:], bg[:, :])
                b1_sb = constp.tile([128, NMT], F32)
                nc.scalar.dma_start(b1_sb[:], b1s[:, :])
                b2_sb = constp.tile([128, N2 // 16], DT)
                nc.scalar.dma_start(b2_sb[:], b2s[:, :])
                ident8 = constp.tile([64, 64], F8)
                make_identity(nc, ident8[:])

                # Weight stream, all on SP, chained at depth 2. Seed the
                # chain with the last two ats loads so W1 cannot jump the
                # queue ahead of the phase-A-critical adjacency stream.
                chain = [at_dmas[-2], at_dmas[-1]]

                def _w_dma(dst_ap, src_ap, depth=2):
                    h = nc.sync.dma_start(dst_ap, src_ap)
                    if len(chain) >= depth:
                        add_dep_helper(h.ins, chain[-depth].ins, sync=True,
                                       reason="weight-stream chain")
                    chain.append(h)
                    return h

                # W1 packed: persist fully ([128, 30, 1600] fp8), 10 chunks
                # (small enough that mid-stream DMAs see bounded queue delay).
                w1sb = w1pool.tile([128, GKT, HS], F8)
                for i in range(10):
                    _w_dma(
                        w1sb[:, 3 * i:3 * (i + 1), :],
                        w1p[3 * i * 128:3 * (i + 1) * 128, :].rearrange(
                            "(a b) c -> b a c", a=3))

                # W2 last-64 k rows, all columns, one DMA; then the 4 column
                # chunks 2x[128, 6, 1600] each, all persistent.
                w2last = w2lpool.tile([64, N2], F8)
                _w_dma(w2last[:], w2s[1536:1600, :])
                w2t = []
                for c in range(4):
                    c0 = c * HS
                    t = w2pool.tile([128, 2 * W2_PAIRS, HS], F8,
                                    tag=f"w2_{c}", bufs=1)
                    for s in range(4):
                        _w_dma(t[:, 3 * s:3 * (s + 1), :],
                               w2s[384 * s:384 * (s + 1),
                                   c0:c0 + HS].rearrange(
                                   "(a b) c -> b a c", a=3))
                    w2t.append(t)

                # Persistent SBUF tensors.
                zT = persist.tile([128, DPC], DT)
                gsb = persist.tile([BS, GPC * NBLK * BS], F8)   # packed G blocks
                gT_big = persist.tile([128, GKT * 64], F8)
                y1T_big = persist.tile([128, NMT * 64], F8)

                # DRAM bounce buffers for the collectives.
                g_loc = dramp.tile([GPC, GP], F8, name="g_loc")
                g_all = dramp.tile([N_GRAPHS, GP], F8, addr_space="Shared",
                                   name="g_all")
                y_loc = dramp.tile([N_GRAPHS, N2], DT, name="y_loc")
                y_red = dramp.tile([GPC, N2], DT, name="y_red")

                # ---- Phase A, reassociated: xtC = xs^T @ C (fp8 DoubleRow,
                # dinv_src folded into xs on host), agg = Wg^T @ xtC ----
                xtc = [xtps.tile([128, DPC], F32, name=f"xtc_ps{m}")
                       for m in range(2)]
                for p in range(KP_NODES):
                    a_t = at[p // 2]
                    a2 = 2 * (p % 2)
                    st = (p == 0)
                    sp = (p == KP_NODES - 1)
                    for m in range(2):
                        lhsT = xsb[:, p, :, m * 128:(m + 1) * 128]
                        for (n0, nw) in ((0, 512), (512, 128)):
                            nc.tensor.matmul(
                                xtc[m][:, n0:n0 + nw], lhsT=lhsT,
                                rhs=a_t[:, a2:a2 + 2, n0:n0 + nw],
                                perf_mode=DR, start=st, stop=sp)
                xtc_sb = [xtcp.tile([128, DPC], DT, tag=f"xtc{m}", bufs=1,
                                    name=f"xtc_sb{m}")
                          for m in range(2)]
                nc.vector.tensor_copy(xtc_sb[0][:], xtc[0][:])
                nc.scalar.activation(xtc_sb[1][:], xtc[1][:],
                                     mybir.ActivationFunctionType.Copy)
                agg = aggps.tile([128, DPC], F32)
                for i in range(2):
                    for (n0, nw) in ((0, 512), (512, 128)):
                        nc.tensor.matmul(agg[:, n0:n0 + nw],
                                         lhsT=wg_sb[:, i, :],
                                         rhs=xtc_sb[i][:, n0:n0 + nw],
                                         start=(i == 0), stop=(i == 1))
                # dinv_dst * agg then relu(+bg), pipelined per graph pair
                # (160 cols) so the G-pack can start on early graphs
                aggs = xtcp.tile([128, DPC], F32, tag="aggs", bufs=1)
                for (h0, hw_) in ((0, 320), (320, 320)):
                    nc.vector.tensor_tensor(aggs[:, h0:h0 + hw_],
                                            agg[:, h0:h0 + hw_],
                                            dinv_d_sb[:, h0:h0 + hw_],
                                            op=mybir.AluOpType.mult)
                    nc.scalar.activation(zT[:, h0:h0 + hw_],
                                         aggs[:, h0:h0 + hw_],
                                         mybir.ActivationFunctionType.Relu,
                                         bias=bg_sb[:, 0:1])

            # ---- Phase B: symmetric-packed G blocks, fp8, one AllGather ----
            with tc.tile_pool(name="gps", bufs=4, space="PSUM") as gps:
                for g in range(GPC):
                    gp = gps.tile([BS, NBLK * BS], F32)
                    zg0 = g * N_NEURONS
                    for b, (i, j) in enumerate(BLOCKS):
                        nc.tensor.matmul(
                            gp[:, b * BS:(b + 1) * BS],
                            lhsT=zT[:, zg0 + i * BS:zg0 + (i + 1) * BS],
                            rhs=zT[:, zg0 + j * BS:zg0 + (j + 1) * BS],
                            start=True, stop=True)
                    dst = gsb[:, g * NBLK * BS:(g + 1) * NBLK * BS]
                    if g % 2 == 0:
                        nc.vector.tensor_copy(dst, gp[:])
                    else:
                        nc.scalar.activation(
                            dst, gp[:], mybir.ActivationFunctionType.Copy)
                    if g in (3, 7):
                        # g_loc[g, b*256+r*16+c] = gsb[r, g*240+b*16+c];
                        # half-writes on the Act HWDGE ring (fast desc-gen;
                        # Act SEQ has nothing pending until post-AG)
                        g0 = g - 3
                        nc.gpsimd.dma_start(
                            g_loc[g0:g0 + 4, :].rearrange(
                                "g (b r c) -> r g b c", b=NBLK, r=BS),
                            gsb[:, g0 * NBLK * BS:(g0 + 4) * NBLK * BS].rearrange(
                                "r (g b c) -> r g b c", g=4, b=NBLK))
                nc.gpsimd.collective_compute(
                    "AllGather", mybir.AluOpType.bypass, replica_groups=RG,
                    ins=[g_loc.opt()], outs=[g_all.opt()],
                )

            # ---- Phase C0: transpose packed G into [128 x 64] K-tiles ----
            with (
                tc.tile_pool(name="gallp", bufs=1) as gallp,
                tc.tile_pool(name="tps", bufs=4, space="PSUM") as tps,
            ):
                ga = gallp.tile([N_GRAPHS, GP], F8)
                nc.gpsimd.dma_start(ga[:, 0:GP // 2], g_all[:, 0:GP // 2])
                nc.gpsimd.dma_start(ga[:, GP // 2:GP], g_all[:, GP // 2:GP])
                for tt in range(GKT // 2):
                    # fp8 PE transpose requires output element step 2: write
                    # into strided PSUM views, read back with the same stride.
                    # Two transposes share one PSUM tile -> one copy per pair.
                    tp = tps.tile([128, 4 * N_GRAPHS], F8)
                    tp_v = tp[:].rearrange("p (n s) -> p n s", s=2)[:, :, 0:1]
                    for j in range(2):
                        t = 2 * tt + j
                        nc.tensor.transpose(
                            tp_v[:, j * 64:(j + 1) * 64],
                            ga[:, t * 128:(t + 1) * 128], ident8[:])
                    dst = gT_big[:, tt * 128:(tt + 1) * 128].rearrange(
                        "p (n s) -> p n s", s=1)
                    if tt % 2 == 0:
                        nc.vector.tensor_copy(dst, tp_v)
                    else:
                        nc.scalar.activation(
                            dst, tp_v, mybir.ActivationFunctionType.Copy)

                # ---- Phase C1: y1T = relu(W1p.T @ Gpacked + 64*b1), fp8 out,
                #      DoubleRow (both operands fp8), W1 stationary ----
                with tc.tile_pool(name="y1ps", bufs=4, space="PSUM") as y1psp:
                    for t, (m0, mw) in enumerate(M_TILES_HS):
                        p1 = y1psp.tile([128, N_GRAPHS], F32)
                        for q in range(GKP):
                            lhsT = w1sb[:, 2 * q:2 * q + 2, m0:m0 + mw]
                            rhs = gT_big[:, q * 128:(q + 1) * 128].rearrange(
                                "p (i n) -> p i n", i=2)
                            nc.tensor.matmul(p1[0:mw, :], lhsT=lhsT, rhs=rhs,
                                             perf_mode=DR,
                                             start=(q == 0), stop=(q == GKP - 1))
                        dst = y1T_big[0:mw, t * 64:(t + 1) * 64]
                        if t % 2 == 0:
                            nc.scalar.activation(
                                dst, p1[0:mw, :],
                                mybir.ActivationFunctionType.Relu,
                                bias=b1_sb[0:mw, t:t + 1])
                        else:
                            # relu on DVE: (x + b1) max 0
                            nc.vector.tensor_scalar(
                                dst, p1[0:mw, :], b1_sb[0:mw, t:t + 1], 0.0,
                                op0=mybir.AluOpType.add,
                                op1=mybir.AluOpType.max)

            # ---- Phase C2: fc2 partials graph-major (y1T stationary fp8,
            #      W2 moving fp8 DoubleRow); one ReduceScatter(add) ----
            with (
                tc.tile_pool(name="p2ps", bufs=2, space="PSUM") as p2psp,
                tc.tile_pool(name="y2sb", bufs=2) as y2sbp,
                tc.tile_pool(name="sig", bufs=1) as sigp,
            ):
                for c in range(4):
                    c0 = c * HS
                    p2 = p2psp.tile([N_GRAPHS, HS], F32)
                    for q in range(W2_PAIRS):
                        lhsT = y1T_big[:, q * 128:(q + 1) * 128].rearrange(
                            "p (i n) -> p i n", i=2)
                        for (n0, nw) in N_CHUNKS_1600:
                            nc.tensor.matmul(
                                p2[:, n0:n0 + nw], lhsT=lhsT,
                                rhs=w2t[c][:, 2 * q:2 * q + 2, n0:n0 + nw],
                                perf_mode=DR, start=(q == 0), stop=False)
                    lhsT_l = y1T_big[0:64, (NMT - 1) * 64:NMT * 64]
                    for (n0, nw) in N_CHUNKS_1600:
                        nc.tensor.matmul(p2[:, n0:n0 + nw], lhsT=lhsT_l,
                                         rhs=w2last[0:64, c0 + n0:c0 + n0 + nw],
                                         start=False, stop=True)
                    y2sb = y2sbp.tile([N_GRAPHS, HS], DT)
                    nc.vector.tensor_copy(y2sb[:], p2[:])
                    nc.gpsimd.dma_start(y_loc[:, c0:c0 + HS], y2sb[:])
                nc.gpsimd.collective_compute(
                    "ReduceScatter", mybir.AluOpType.add, replica_groups=RG,
                    ins=[y_loc.opt()], outs=[y_red.opt()],
                )
                # tail: sigmoid((P2 + 4096*b2)/4096), [8, 6400] -> 2 x
                # [128, 200] halves pipelined across engines
                w16 = N2 // 32
                for h in range(2):
                    ys = sigp.tile([128, w16], DT, tag=f"ys{h}", bufs=1,
                                   name=f"ys{h}")
                    eng = nc.sync if h == 0 else nc.scalar
                    eng.dma_start(
                        ys[:], y_red[:, h * 3200:(h + 1) * 3200].rearrange(
                            "g (j t) -> g j t", j=16))
                    yb = sigp.tile([128, w16], F32, tag=f"yb{h}", bufs=1,
                                   name=f"yb{h}")
                    nc.vector.tensor_tensor(
                        yb[:], ys[:], b2_sb[:, h * w16:(h + 1) * w16],
                        op=mybir.AluOpType.add)
                    yo = sigp.tile([128, w16], F32, tag=f"yo{h}", bufs=1,
                                   name=f"yo{h}")
                    nc.scalar.activation(yo[:], yb[:],
                                         mybir.ActivationFunctionType.Sigmoid,
                                         scale=1.0 / P2SCALE)
                    nc.sync.dma_start(
                        y[:, h * (N2 // 2):(h + 1) * (N2 // 2)].rearrange(
                            "g (j t) -> g j t", j=16), yo[:])

    _fix_excess_waits(nc)
    return nc


_NC_CACHE = None


def _get_nc():
    global _NC_CACHE
    if _NC_CACHE is None:
        _NC_CACHE = build_nc()
    return _NC_CACHE


def prep_in_maps(x, edge_index, Wg, bg, W1, b1, W2, b2):
    x = np.asarray(x, np.float32)
    edge_index = np.asarray(edge_index)
    Wg = np.asarray(Wg, np.float32)
    bg = np.asarray(bg, np.float32)
    W1 = np.asarray(W1, np.float32)
    b1 = np.asarray(b1, np.float32)
    W2 = np.asarray(W2, np.float32)
    b2 = np.asarray(b2, np.float32)

    src = edge_index[0].astype(np.int64)
    dst = edge_index[1].astype(np.int64)

    deg = np.bincount(dst, minlength=N_NODES).astype(np.float32)
    dinv = np.where(deg > 0, 1.0 / np.sqrt(np.maximum(deg, 1.0)), 0.0).astype(np.float32)

    # Dense edge-count matrix [src, dst]; counts are small ints, exact in fp8.
    counts = np.bincount(src * N_NODES + dst, minlength=N_NODES * N_NODES)
    at_full = counts.astype(NP_F8).reshape(N_NODES, N_NODES)

    # xs: dinv_src-prescaled x in fp8, packed [(p), (pair, i, tbl)] so each
    # [:, pair, :, :] slice is a DoubleRow lhsT of two node-tiles.
    xs8 = (x * dinv[:, None]).astype(NP_F8)
    xs_np = np.ascontiguousarray(
        xs8.reshape(KP_NODES, 2, 128, TBL).transpose(2, 0, 1, 3).reshape(
            128, KP_NODES * 2 * TBL))
    wg_np = Wg.astype(NP_BF16)
    bg_np = np.ascontiguousarray(bg.reshape(LATENT, 1))

    NMT = len(M_TILES_HS)
    # b1 per-core slices, scaled x64, laid out [128 partitions, m-tile].
    # b2 scaled x4096, laid out per RS half: [(g j) partitions, t] matching
    # each [8, 3200] -> [128, 200] reshape (same for every g).
    b2q = (b2 * P2SCALE).astype(NP_BF16)
    halves = [np.tile(b2q[h * 3200:(h + 1) * 3200].reshape(16, 200), (8, 1))
              for h in range(2)]
    b2_np = np.ascontiguousarray(np.concatenate(halves, axis=1))

    in_maps = []
    for c in range(N_CORES):
        s0 = c * HS
        W1c = W1[:, s0:s0 + HS].reshape(NB, BS, NB, BS, HS)  # [i, r, j, c, m]
        w1p_np = np.empty((NBLK, BS, BS, HS), np.float32)
        for b, (i, j) in enumerate(BLOCKS):
            blk = W1c[i, :, j, :, :]
            if i != j:
                blk = blk + W1c[j, :, i, :, :].transpose(1, 0, 2)
            w1p_np[b] = blk
        w1p_np = (w1p_np.reshape(GP, HS) * W1SCALE).astype(NP_F8)

        b1c = b1[s0:s0 + HS] * W1SCALE
        b1_np = np.zeros((128, NMT), np.float32)
        for t, (m0, mw) in enumerate(M_TILES_HS):
            b1_np[0:mw, t] = b1c[m0:m0 + mw]

        in_maps.append({
            "xs": xs_np,
            "wg": wg_np,
            "bg": bg_np,
            "ats": np.ascontiguousarray(at_full[:, c * DPC:(c + 1) * DPC]),
            "dinv_d": np.ascontiguousarray(np.broadcast_to(
                dinv[c * DPC:(c + 1) * DPC], (128, DPC))),
            "w1p": w1p_np,
            "b1s": b1_np,
            "w2s": np.ascontiguousarray(
                (W2[s0:s0 + HS, :] * W2SCALE).astype(NP_F8)),
            "b2s": b2_np,
        })
    return in_maps


def kernel(x, edge_index, Wg, bg, W1, b1, W2, b2):
    in_maps = prep_in_maps(x, edge_index, Wg, bg, W1, b1, W2, b2)
    nc = _get_nc()
    res = run_bass_kernel_spmd(nc, in_maps, core_ids=list(range(N_CORES)))
    out = np.concatenate([res.results[c]["y"] for c in range(N_CORES)], axis=0)
    return out.reshape(-1).astype(np.float32)
